# revision 9
# baseline (speedup 1.0000x reference)
"""DoubleGRU Trainium2 Bass kernel.

Strategy
--------
Data-parallel over batch across 8 NeuronCores. All activations live in
[feature=128 partitions, batch (free dim)] layout on-device so every matmul
streams activation columns through a stationary [128,128] weight (out =
W.T @ actT = (act @ W).T) with zero on-device transposes; the layout change
(and the f32<->bf16 conversion) happens on the host before upload / after
download.

Engine balance: ScalarE (ACT) is the structural bottleneck (4 sigmoid + 2
tanh per element, 1 elem/cycle/lane). So: x/old_h/weights arrive as bf16
(no on-device casts, half the DMA), the z|r gate pair of each stage is one
fused [128, 2*CH] sigmoid call, relu runs on VectorE, and the GRU combines
are split between VectorE (critical path) and GpSimd (off-path). PSUM
accumulates the x-path and h-path matmuls of each gate; ACT reads PSUM
directly. DMA moves 4-chunk groups (4 KiB/partition) on the HWDGE ring.
"""
import os
import sys

sys.path.insert(0, "/opt/trn_rl_repo")

import numpy as np
import ml_dtypes

import concourse.bass as bass
import concourse.tile as tile
from concourse import bacc, bass_utils, mybir

B = 131072
S = 128
NCORES = 8
BC = B // NCORES  # 16384 batch rows per core
CH = 512          # batch columns per compute chunk (one PSUM bank)
NCH = BC // CH
GRP = int(os.environ.get("K_GRP", "4"))  # compute chunks per DMA transfer
DCH = CH * GRP

F32 = mybir.dt.float32
BF16 = mybir.dt.bfloat16
NPBF16 = ml_dtypes.bfloat16
AF = mybir.ActivationFunctionType

_NC_CACHE = {}

BUFS_ACTS = int(os.environ.get("K_BUFS_ACTS", "3"))
BUFS_INP = int(os.environ.get("K_BUFS_INP", "4"))
BUFS_OUT = int(os.environ.get("K_BUFS_OUT", "3"))
VARIANT = os.environ.get("K_VARIANT", "full")


def _build(use_bias: bool, repeat: int = 1, compile: bool = True):
    nc = bacc.Bacc("TRN2", target_bir_lowering=False, debug=False, num_devices=NCORES)

    xT = nc.dram_tensor("xT", [S, BC], BF16, kind="ExternalInput").ap()
    ohT = nc.dram_tensor("ohT", [S, BC], BF16, kind="ExternalInput").ap()
    wx1 = nc.dram_tensor("wx1", [3, S, S], BF16, kind="ExternalInput").ap()
    wx2 = nc.dram_tensor("wx2", [3, S, S], BF16, kind="ExternalInput").ap()
    wh = nc.dram_tensor("wh", [6, S, S], BF16, kind="ExternalInput").ap()
    bT = nc.dram_tensor("bT", [6, S, 1], F32, kind="ExternalInput").ap()
    midw = nc.dram_tensor("midw", [S, S], BF16, kind="ExternalInput").ap()
    outT = nc.dram_tensor("outT", [S, BC], BF16, kind="ExternalOutput").ap()

    with tile.TileContext(nc) as tc:
        with (
            tc.tile_pool(name="wpool", bufs=1) as wp,
            tc.tile_pool(name="inp", bufs=BUFS_INP) as inp,
            tc.tile_pool(name="acts", bufs=BUFS_ACTS) as acts,
            tc.tile_pool(name="scr", bufs=int(os.environ.get("K_BUFS_SCR", "2"))) as scr,
            tc.tile_pool(name="outp", bufs=BUFS_OUT) as outp,
            tc.tile_pool(name="ps", bufs=1, space="PSUM") as psp,
            tc.tile_pool(name="ps2", bufs=2, space="PSUM") as psp2,
        ):
            def load_w(src, tag):
                t = wp.tile([S, S], BF16, tag=tag)
                nc.sync.dma_start(out=t, in_=src)
                return t

            w_x1 = [load_w(wx1[i, :, :], f"wx1{i}") for i in range(3)]
            w_x2 = [load_w(wx2[i, :, :], f"wx2{i}") for i in range(3)]
            w_h = [load_w(wh[i, :, :], f"wh{i}") for i in range(6)]
            w_mid = load_w(midw, "wmid")

            b_t = []
            if use_bias:
                for i in range(6):
                    t = wp.tile([S, 1], F32, tag=f"b{i}")
                    nc.sync.dma_start(out=t, in_=bT[i, :, :])
                    b_t.append(t)

            def act(out_ap, in_ap, func, bias_idx):
                if use_bias:
                    nc.scalar.activation(out_ap, in_ap, func, bias=b_t[bias_idx][:])
                else:
                    nc.scalar.activation(out_ap, in_ap, func)

            if VARIANT == "gpschain":
                cur = wp.tile([S, CH], BF16, tag="xs")
                nc.vector.memset(cur, 0.25)
                for it in range(NCH * repeat):
                    zc = acts.tile([S, CH], BF16, tag="zc")
                    nc.gpsimd.tensor_scalar(zc[:], cur[:], -1.0, 1.0,
                                            mybir.AluOpType.mult, mybir.AluOpType.add)
                    m1 = acts.tile([S, CH], BF16, tag="m1")
                    nc.gpsimd.tensor_mul(m1[:], zc[:], zc[:])
                    z2c = acts.tile([S, CH], BF16, tag="z2c")
                    nc.gpsimd.tensor_scalar(z2c[:], m1[:], -1.0, 1.0,
                                            mybir.AluOpType.mult, mybir.AluOpType.add)
                    m3 = acts.tile([S, CH], BF16, tag="m3")
                    nc.gpsimd.tensor_mul(m3[:], z2c[:], z2c[:])
                    cur = m3
                hf = outp.tile([S, DCH], BF16, tag="h")
                nc.vector.memset(hf, 0.0)
                nc.vector.tensor_copy(hf[:, 0:CH], cur[:])
                nc.sync.dma_start(out=outT[:, 0:DCH], in_=hf[:])

            if VARIANT == "dvechain":
                cur = wp.tile([S, CH], BF16, tag="xs")
                nc.vector.memset(cur, 0.25)
                for it in range(NCH * repeat):
                    for j in range(9):
                        tt = acts.tile([S, CH], BF16, tag=f"e{j}")
                        nc.vector.tensor_mul(tt[:], cur[:], cur[:])
                        cur = tt
                hf = outp.tile([S, DCH], BF16, tag="h")
                nc.vector.memset(hf, 0.0)
                nc.vector.tensor_copy(hf[:, 0:CH], cur[:])
                nc.sync.dma_start(out=outT[:, 0:DCH], in_=hf[:])

            if VARIANT in ("act2bank", "act1bank"):
                cur = wp.tile([S, 2 * CH], BF16, tag="xs")
                nc.vector.memset(cur, 0.25)
                for it in range(NCH * repeat):
                    ps_zr = psp.tile([S, 2 * CH], F32, tag="ps_zr")
                    nc.tensor.matmul(ps_zr[:, :CH], w_x1[0][:], cur[:, :CH], start=True, stop=False)
                    nc.tensor.matmul(ps_zr[:, :CH], w_h[0][:], cur[:, CH:], start=False, stop=True)
                    nc.tensor.matmul(ps_zr[:, CH:], w_x1[1][:], cur[:, :CH], start=True, stop=False)
                    nc.tensor.matmul(ps_zr[:, CH:], w_h[1][:], cur[:, CH:], start=False, stop=True)
                    zr = acts.tile([S, 2 * CH], BF16, tag="zr")
                    if VARIANT == "act2bank":
                        nc.scalar.activation(zr[:], ps_zr[:], AF.Sigmoid)
                    else:
                        nc.scalar.activation(zr[:, :CH], ps_zr[:, :CH], AF.Sigmoid)
                        nc.scalar.activation(zr[:, CH:], ps_zr[:, CH:], AF.Sigmoid)
                    cur = zr
                hf = outp.tile([S, DCH], BF16, tag="h")
                nc.vector.memset(hf, 0.0)
                nc.vector.tensor_copy(hf[:, 0:CH], cur[:, :CH])
                nc.sync.dma_start(out=outT[:, 0:DCH], in_=hf[:])

            if VARIANT == "gps4":
                xs = wp.tile([S, CH], BF16, tag="xs")
                nc.vector.memset(xs, 0.25)
                for it in range(NCH * repeat):
                    zc = acts.tile([S, CH], BF16, tag="zc")
                    nc.gpsimd.tensor_scalar(zc[:], xs[:], -1.0, 1.0,
                                            mybir.AluOpType.mult, mybir.AluOpType.add)
                    m1 = acts.tile([S, CH], BF16, tag="m1")
                    nc.gpsimd.tensor_mul(m1[:], xs[:], xs[:])
                    z2c = acts.tile([S, CH], BF16, tag="z2c")
                    nc.gpsimd.tensor_scalar(z2c[:], xs[:], -1.0, 1.0,
                                            mybir.AluOpType.mult, mybir.AluOpType.add)
                    m3 = acts.tile([S, CH], BF16, tag="m3")
                    nc.gpsimd.tensor_mul(m3[:], xs[:], xs[:])
                hf = outp.tile([S, DCH], BF16, tag="h")
                nc.vector.memset(hf, 0.0)
                nc.sync.dma_start(out=outT[:, 0:DCH], in_=hf[:])

            if VARIANT == "dve9":
                xs = wp.tile([S, CH], BF16, tag="xs")
                nc.vector.memset(xs, 0.25)
                for it in range(NCH * repeat):
                    for j in range(9):
                        tt = acts.tile([S, CH], BF16, tag=f"e{j}")
                        nc.vector.tensor_mul(tt[:], xs[:], xs[:])
                hf = outp.tile([S, DCH], BF16, tag="h")
                nc.vector.memset(hf, 0.0)
                nc.sync.dma_start(out=outT[:, 0:DCH], in_=hf[:])

            if VARIANT == "mm":
                xs = wp.tile([S, CH], BF16, tag="xs")
                nc.vector.memset(xs, 0.25)
                ohs = wp.tile([S, CH], BF16, tag="ohs")
                nc.vector.memset(ohs, 0.25)
                for it in range(NCH * repeat):
                    ps_zr = psp.tile([S, 2 * CH], F32, tag="ps_zr")
                    nc.tensor.matmul(ps_zr[:, :CH], w_x1[0][:], xs[:], start=True, stop=False)
                    nc.tensor.matmul(ps_zr[:, :CH], w_h[0][:], ohs[:], start=False, stop=True)
                    nc.tensor.matmul(ps_zr[:, CH:], w_x1[1][:], xs[:], start=True, stop=False)
                    nc.tensor.matmul(ps_zr[:, CH:], w_h[1][:], ohs[:], start=False, stop=True)
                    ps_ht = psp2.tile([S, CH], F32, tag="ps_ht")
                    nc.tensor.matmul(ps_ht[:], w_x1[2][:], xs[:], start=True, stop=False)
                    nc.tensor.matmul(ps_ht[:], w_h[2][:], ohs[:], start=False, stop=True)
                    ps_mid = psp.tile([S, CH], F32, tag="ps_mid")
                    nc.tensor.matmul(ps_mid[:], w_mid[:], ohs[:], start=True, stop=True)
                    ps_zr2 = psp.tile([S, 2 * CH], F32, tag="ps_zr2")
                    nc.tensor.matmul(ps_zr2[:, :CH], w_x2[0][:], xs[:], start=True, stop=False)
                    nc.tensor.matmul(ps_zr2[:, :CH], w_h[0][:], ohs[:], start=False, stop=True)
                    nc.tensor.matmul(ps_zr2[:, CH:], w_x2[1][:], xs[:], start=True, stop=False)
                    nc.tensor.matmul(ps_zr2[:, CH:], w_h[4][:], ohs[:], start=False, stop=True)
                    ps_ht2 = psp.tile([S, CH], F32, tag="ps_ht2")
                    nc.tensor.matmul(ps_ht2[:], w_x2[2][:], xs[:], start=True, stop=False)
                    nc.tensor.matmul(ps_ht2[:], w_h[5][:], ohs[:], start=False, stop=True)
                hf = outp.tile([S, DCH], BF16, tag="h")
                nc.vector.memset(hf, 0.0)
                nc.sync.dma_start(out=outT[:, 0:DCH], in_=hf[:])

            if VARIANT == "mmact":
                xs = wp.tile([S, CH], BF16, tag="xs")
                nc.vector.memset(xs, 0.25)
                ohs = wp.tile([S, CH], BF16, tag="ohs")
                nc.vector.memset(ohs, 0.25)
                for it in range(NCH * repeat):
                    ps_zr = psp.tile([S, 2 * CH], F32, tag="ps_zr")
                    nc.tensor.matmul(ps_zr[:, :CH], w_x1[0][:], xs[:], start=True, stop=False)
                    nc.tensor.matmul(ps_zr[:, :CH], w_h[0][:], ohs[:], start=False, stop=True)
                    nc.tensor.matmul(ps_zr[:, CH:], w_x1[1][:], xs[:], start=True, stop=False)
                    nc.tensor.matmul(ps_zr[:, CH:], w_h[1][:], ohs[:], start=False, stop=True)
                    zr = acts.tile([S, 2 * CH], BF16, tag="zr")
                    nc.scalar.activation(zr[:], ps_zr[:], AF.Sigmoid)
                    ps_ht = psp2.tile([S, CH], F32, tag="ps_ht")
                    nc.tensor.matmul(ps_ht[:], w_x1[2][:], xs[:], start=True, stop=False)
                    nc.tensor.matmul(ps_ht[:], w_h[2][:], ohs[:], start=False, stop=True)
                    ht = acts.tile([S, CH], BF16, tag="ht")
                    nc.scalar.activation(ht[:], ps_ht[:], AF.Tanh)
                    ps_mid = psp.tile([S, CH], F32, tag="ps_mid")
                    nc.tensor.matmul(ps_mid[:], w_mid[:], ohs[:], start=True, stop=True)
                    mx = acts.tile([S, CH], BF16, tag="mid_x")
                    nc.vector.tensor_scalar_max(mx[:], ps_mid[:], 0.0)
                    ps_zr2 = psp.tile([S, 2 * CH], F32, tag="ps_zr2")
                    nc.tensor.matmul(ps_zr2[:, :CH], w_x2[0][:], xs[:], start=True, stop=False)
                    nc.tensor.matmul(ps_zr2[:, :CH], w_h[0][:], ohs[:], start=False, stop=True)
                    nc.tensor.matmul(ps_zr2[:, CH:], w_x2[1][:], xs[:], start=True, stop=False)
                    nc.tensor.matmul(ps_zr2[:, CH:], w_h[4][:], ohs[:], start=False, stop=True)
                    zr2 = acts.tile([S, 2 * CH], BF16, tag="zr2")
                    nc.scalar.activation(zr2[:], ps_zr2[:], AF.Sigmoid)
                    ps_ht2 = psp.tile([S, CH], F32, tag="ps_ht2")
                    nc.tensor.matmul(ps_ht2[:], w_x2[2][:], xs[:], start=True, stop=False)
                    nc.tensor.matmul(ps_ht2[:], w_h[5][:], ohs[:], start=False, stop=True)
                    ht2 = acts.tile([S, CH], BF16, tag="ht2")
                    nc.scalar.activation(ht2[:], ps_ht2[:], AF.Tanh)
                hf = outp.tile([S, DCH], BF16, tag="h")
                nc.vector.memset(hf, 0.0)
                nc.sync.dma_start(out=outT[:, 0:DCH], in_=hf[:])

            if VARIANT == "full":
                # group-granular 2-stage software pipeline over groups of 4
                # chunks; ACT calls pair-width (1024 cols), GRU combines
                # quad-width. Sig and tanh emission is split so that between
                # a pair's r-sigmoid and its tanh (whose input chains through
                # rh on VectorE + 4 matmuls) the OTHER stage's sigmoids keep
                # ScalarE busy.
                PAIR = 2 * CH
                QUAD = 4 * CH
                NG = NCH // GRP
                st1 = {}
                st2 = {}
                dmas = {}
                GPS_SUB = os.environ.get("K_GPS_SUB", "1") == "1"

                def dma_group(g):
                    gd = g % NG
                    gs = bass.ts(gd, DCH)
                    x4 = inp.tile([S, DCH], BF16, tag="x")
                    nc.sync.dma_start(out=x4, in_=xT[:, gs])
                    oh4 = inp.tile([S, DCH], BF16, tag="oh")
                    nc.sync.dma_start(out=oh4, in_=ohT[:, gs])
                    dmas[g] = (gs, x4, oh4)

                def s1_sig(g, q):
                    if q == 0:
                        gs, x4, oh4 = dmas.pop(g)
                        zq = acts.tile([S, QUAD], BF16, tag="zq")
                        rq = acts.tile([S, QUAD], BF16, tag="rq")
                        htq = acts.tile([S, QUAD], BF16, tag="htq")
                        st1[g] = dict(gs=gs, x4=x4, oh4=oh4, zq=zq, rq=rq,
                                      htq=htq, rh=[None, None])
                    s = st1[g]
                    x4, oh4 = s["x4"], s["oh4"]
                    u = 2 * q
                    pr = slice(u * CH, (u + 2) * CH)
                    cs = [slice((u + k) * CH, (u + k + 1) * CH) for k in (0, 1)]

                    pza = psp.tile([S, PAIR], F32, tag="pza")
                    for k in (0, 1):
                        o = slice(k * CH, (k + 1) * CH)
                        nc.tensor.matmul(pza[:, o], w_x1[0][:], x4[:, cs[k]], start=True, stop=False)
                        nc.tensor.matmul(pza[:, o], w_h[0][:], oh4[:, cs[k]], start=False, stop=True)
                    act(s["zq"][:, pr], pza[:], AF.Sigmoid, 0)

                    pzb = psp.tile([S, PAIR], F32, tag="pzb")
                    for k in (0, 1):
                        o = slice(k * CH, (k + 1) * CH)
                        nc.tensor.matmul(pzb[:, o], w_x1[1][:], x4[:, cs[k]], start=True, stop=False)
                        nc.tensor.matmul(pzb[:, o], w_h[1][:], oh4[:, cs[k]], start=False, stop=True)
                    act(s["rq"][:, pr], pzb[:], AF.Sigmoid, 1)

                    rh = scr.tile([S, PAIR], BF16, tag="rh")
                    nc.vector.tensor_mul(rh[:], s["rq"][:, pr], oh4[:, pr])
                    s["rh"][q] = rh

                def s1_tanh(g, q):
                    s = st1[g]
                    x4, rh = s["x4"], s["rh"][q]
                    u = 2 * q
                    pr = slice(u * CH, (u + 2) * CH)
                    cs = [slice((u + k) * CH, (u + k + 1) * CH) for k in (0, 1)]
                    pt = psp.tile([S, PAIR], F32, tag="pt")
                    for k in (0, 1):
                        o = slice(k * CH, (k + 1) * CH)
                        nc.tensor.matmul(pt[:, o], w_x1[2][:], x4[:, cs[k]], start=True, stop=False)
                        nc.tensor.matmul(pt[:, o], w_h[2][:], rh[:, o], start=False, stop=True)
                    act(s["htq"][:, pr], pt[:], AF.Tanh, 2)

                def s1_quad(g):
                    s = st1.pop(g)
                    oh4, zq, htq = s["oh4"], s["zq"], s["htq"]
                    dq = scr.tile([S, QUAD], BF16, tag="dq")
                    if GPS_SUB:
                        nc.gpsimd.tensor_sub(dq[:], oh4[:], htq[:])
                    else:
                        nc.vector.tensor_sub(dq[:], oh4[:], htq[:])
                    pq = scr.tile([S, QUAD], BF16, tag="pq")
                    nc.vector.tensor_mul(pq[:], zq[:], dq[:])
                    mid_hq = acts.tile([S, QUAD], BF16, tag="mid_hq")
                    nc.vector.tensor_add(mid_hq[:], htq[:], pq[:])
                    st2[g] = dict(gs=s["gs"], mid_hq=mid_hq, r2h=[None, None])

                def s2_sig(g, q):
                    s = st2[g]
                    if q == 0:
                        mxq = acts.tile([S, QUAD], BF16, tag="mxq")
                        z2q = acts.tile([S, QUAD], BF16, tag="z2q")
                        r2q = acts.tile([S, QUAD], BF16, tag="r2q")
                        ht2q = acts.tile([S, QUAD], BF16, tag="ht2q")
                        s.update(mxq=mxq, z2q=z2q, r2q=r2q, ht2q=ht2q)
                    mid_hq, mxq = s["mid_hq"], s["mxq"]
                    u = 2 * q
                    pr = slice(u * CH, (u + 2) * CH)
                    cs = [slice((u + k) * CH, (u + k + 1) * CH) for k in (0, 1)]

                    pm = psp.tile([S, PAIR], F32, tag="pm")
                    for k in (0, 1):
                        o = slice(k * CH, (k + 1) * CH)
                        nc.tensor.matmul(pm[:, o], w_mid[:], mid_hq[:, cs[k]], start=True, stop=True)
                    nc.vector.tensor_scalar_max(mxq[:, pr], pm[:], 0.0)

                    pza = psp.tile([S, PAIR], F32, tag="pza")
                    for k in (0, 1):
                        o = slice(k * CH, (k + 1) * CH)
                        nc.tensor.matmul(pza[:, o], w_x2[0][:], mxq[:, cs[k]], start=True, stop=False)
                        nc.tensor.matmul(pza[:, o], w_h[0][:], mid_hq[:, cs[k]], start=False, stop=True)
                    act(s["z2q"][:, pr], pza[:], AF.Sigmoid, 0)

                    pzb = psp.tile([S, PAIR], F32, tag="pzb")
                    for k in (0, 1):
                        o = slice(k * CH, (k + 1) * CH)
                        nc.tensor.matmul(pzb[:, o], w_x2[1][:], mxq[:, cs[k]], start=True, stop=False)
                        nc.tensor.matmul(pzb[:, o], w_h[4][:], mid_hq[:, cs[k]], start=False, stop=True)
                    act(s["r2q"][:, pr], pzb[:], AF.Sigmoid, 4)

                    r2h = scr.tile([S, PAIR], BF16, tag="r2h")
                    nc.vector.tensor_mul(r2h[:], s["r2q"][:, pr], mid_hq[:, pr])
                    s["r2h"][q] = r2h

                def s2_tanh(g, q):
                    s = st2[g]
                    mxq, r2h = s["mxq"], s["r2h"][q]
                    u = 2 * q
                    pr = slice(u * CH, (u + 2) * CH)
                    cs = [slice((u + k) * CH, (u + k + 1) * CH) for k in (0, 1)]
                    pt = psp.tile([S, PAIR], F32, tag="pt")
                    for k in (0, 1):
                        o = slice(k * CH, (k + 1) * CH)
                        nc.tensor.matmul(pt[:, o], w_x2[2][:], mxq[:, cs[k]], start=True, stop=False)
                        nc.tensor.matmul(pt[:, o], w_h[5][:], r2h[:, o], start=False, stop=True)
                    act(s["ht2q"][:, pr], pt[:], AF.Tanh, 5)

                def s2_quad(g):
                    s = st2.pop(g)
                    mid_hq, z2q, ht2q = s["mid_hq"], s["z2q"], s["ht2q"]
                    d2q = scr.tile([S, QUAD], BF16, tag="d2q")
                    if GPS_SUB:
                        nc.gpsimd.tensor_sub(d2q[:], mid_hq[:], ht2q[:])
                    else:
                        nc.vector.tensor_sub(d2q[:], mid_hq[:], ht2q[:])
                    p2q = scr.tile([S, QUAD], BF16, tag="p2q")
                    nc.vector.tensor_mul(p2q[:], z2q[:], d2q[:])
                    h4 = outp.tile([S, QUAD], BF16, tag="h")
                    nc.vector.tensor_add(h4[:], ht2q[:], p2q[:])
                    nc.sync.dma_start(out=outT[:, s["gs"]], in_=h4[:])

                NGT = NG * repeat
                dma_group(0)
                if NGT > 1:
                    dma_group(1)
                for q in (0, 1):
                    s1_sig(0, q)
                    s1_tanh(0, q)
                s1_quad(0)
                for g in range(NGT):
                    nxt = g + 1 < NGT
                    for q in (0, 1):
                        if nxt:
                            s1_sig(g + 1, q)
                        s2_sig(g, q)
                        if nxt:
                            s1_tanh(g + 1, q)
                        s2_tanh(g, q)
                    if nxt:
                        if g + 2 < NGT:
                            dma_group(g + 2)
                        s1_quad(g + 1)
                    s2_quad(g)

    if compile:
        nc.compile()
    return nc


def _prep_in_maps(x, old_h, W_x1, W_x2, W_h, b, mid):
    """Host-side sharding + layout + dtype prep. Returns per-core input maps."""
    x = np.asarray(x, dtype=np.float32)
    old_h = np.asarray(old_h, dtype=np.float32)
    W_x1 = np.ascontiguousarray(W_x1, dtype=np.float32).astype(NPBF16)
    W_x2 = np.ascontiguousarray(W_x2, dtype=np.float32).astype(NPBF16)
    W_h = np.ascontiguousarray(W_h, dtype=np.float32).astype(NPBF16)
    b = np.asarray(b, dtype=np.float32)
    mid = np.ascontiguousarray(mid, dtype=np.float32).astype(NPBF16)

    xT = np.ascontiguousarray(x.T).astype(NPBF16)      # [S, B]
    ohT = np.ascontiguousarray(old_h.T).astype(NPBF16)
    bTh = np.ascontiguousarray(b.reshape(6, 1, S).transpose(0, 2, 1))  # [6,S,1]

    in_maps = []
    for c in range(NCORES):
        sl = slice(c * BC, (c + 1) * BC)
        in_maps.append({
            "xT": np.ascontiguousarray(xT[:, sl]),
            "ohT": np.ascontiguousarray(ohT[:, sl]),
            "wx1": W_x1,
            "wx2": W_x2,
            "wh": W_h,
            "bT": bTh,
            "midw": mid,
        })
    return in_maps


def kernel(x, old_h, W_x1, W_x2, W_h, b, mid, trace=False):
    assert np.asarray(x).shape == (B, S) and np.asarray(old_h).shape == (B, S)
    b = np.asarray(b, dtype=np.float32)
    use_bias = bool(np.any(b != 0.0))
    key = use_bias
    if key not in _NC_CACHE:
        _NC_CACHE[key] = _build(use_bias)
    nc = _NC_CACHE[key]

    in_maps = _prep_in_maps(x, old_h, W_x1, W_x2, W_h, b, mid)
    res = bass_utils.run_bass_kernel_spmd(
        nc, in_maps, core_ids=list(range(NCORES)), trace=trace
    )
    outT = np.concatenate(
        [res.results[c]["outT"].astype(np.float32) for c in range(NCORES)], axis=1
    )
    h = np.ascontiguousarray(outT.T)
    if trace:
        return (h,), res
    return (h,)


# revision 11
# speedup vs baseline: 1.2522x; 1.2522x over previous
"""DoubleGRU Trainium2 Bass kernel.

Strategy
--------
Data-parallel over batch across 8 NeuronCores. All activations live in
[feature=128 partitions, batch (free dim)] layout on-device so every matmul
streams activation columns through a stationary [128,128] weight (out =
W.T @ actT = (act @ W).T) with zero on-device transposes; the layout change
(and the f32<->bf16 conversion) happens on the host before upload / after
download.

Engine balance: ScalarE (ACT) is the structural bottleneck (4 sigmoid + 2
tanh per element, 1 elem/cycle/lane). So: x/old_h/weights arrive as bf16
(no on-device casts, half the DMA), the z|r gate pair of each stage is one
fused [128, 2*CH] sigmoid call, relu runs on VectorE, and the GRU combines
are split between VectorE (critical path) and GpSimd (off-path). PSUM
accumulates the x-path and h-path matmuls of each gate; ACT reads PSUM
directly. DMA moves 4-chunk groups (4 KiB/partition) on the HWDGE ring.
"""
import os
import sys

sys.path.insert(0, "/opt/trn_rl_repo")

import numpy as np
import ml_dtypes

import concourse.bass as bass
import concourse.tile as tile
from concourse import bacc, bass_utils, mybir

B = 131072
S = 128
NCORES = 8
BC = B // NCORES  # 16384 batch rows per core
CH = 512          # batch columns per compute chunk (one PSUM bank)
NCH = BC // CH
GRP = int(os.environ.get("K_GRP", "4"))  # compute chunks per DMA transfer
DCH = CH * GRP

F32 = mybir.dt.float32
BF16 = mybir.dt.bfloat16
NPBF16 = ml_dtypes.bfloat16
AF = mybir.ActivationFunctionType

_NC_CACHE = {}

BUFS_ACTS = int(os.environ.get("K_BUFS_ACTS", "3"))
BUFS_INP = int(os.environ.get("K_BUFS_INP", "4"))
BUFS_OUT = int(os.environ.get("K_BUFS_OUT", "3"))
VARIANT = os.environ.get("K_VARIANT", "full")


def _build(use_bias: bool, repeat: int = 1, compile: bool = True):
    nc = bacc.Bacc("TRN2", target_bir_lowering=False, debug=False, num_devices=NCORES)

    xT = nc.dram_tensor("xT", [S, BC], BF16, kind="ExternalInput").ap()
    ohT = nc.dram_tensor("ohT", [S, BC], BF16, kind="ExternalInput").ap()
    wx1 = nc.dram_tensor("wx1", [3, S, S], BF16, kind="ExternalInput").ap()
    wx2 = nc.dram_tensor("wx2", [3, S, S], BF16, kind="ExternalInput").ap()
    wh = nc.dram_tensor("wh", [6, S, S], BF16, kind="ExternalInput").ap()
    bT = nc.dram_tensor("bT", [6, S, 1], F32, kind="ExternalInput").ap()
    midw = nc.dram_tensor("midw", [S, S], BF16, kind="ExternalInput").ap()
    outT = nc.dram_tensor("outT", [S, BC], BF16, kind="ExternalOutput").ap()

    with tile.TileContext(nc) as tc:
        with (
            tc.tile_pool(name="wpool", bufs=1) as wp,
            tc.tile_pool(name="inp", bufs=BUFS_INP) as inp,
            tc.tile_pool(name="acts", bufs=BUFS_ACTS) as acts,
            tc.tile_pool(name="scr", bufs=int(os.environ.get("K_BUFS_SCR", "2"))) as scr,
            tc.tile_pool(name="outp", bufs=BUFS_OUT) as outp,
            tc.tile_pool(name="ps", bufs=1, space="PSUM") as psp,
            tc.tile_pool(name="ps2", bufs=2, space="PSUM") as psp2,
        ):
            def load_w(src, tag):
                t = wp.tile([S, S], BF16, tag=tag)
                nc.sync.dma_start(out=t, in_=src)
                return t

            w_x1 = [load_w(wx1[i, :, :], f"wx1{i}") for i in range(3)]
            w_x2 = [load_w(wx2[i, :, :], f"wx2{i}") for i in range(3)]
            w_h = [load_w(wh[i, :, :], f"wh{i}") for i in range(6)]
            w_mid = load_w(midw, "wmid")

            b_t = []
            if use_bias:
                for i in range(6):
                    t = wp.tile([S, 1], F32, tag=f"b{i}")
                    nc.sync.dma_start(out=t, in_=bT[i, :, :])
                    b_t.append(t)

            def act(out_ap, in_ap, func, bias_idx):
                if use_bias:
                    nc.scalar.activation(out_ap, in_ap, func, bias=b_t[bias_idx][:])
                else:
                    nc.scalar.activation(out_ap, in_ap, func)

            if VARIANT == "gpschain":
                cur = wp.tile([S, CH], BF16, tag="xs")
                nc.vector.memset(cur, 0.25)
                for it in range(NCH * repeat):
                    zc = acts.tile([S, CH], BF16, tag="zc")
                    nc.gpsimd.tensor_scalar(zc[:], cur[:], -1.0, 1.0,
                                            mybir.AluOpType.mult, mybir.AluOpType.add)
                    m1 = acts.tile([S, CH], BF16, tag="m1")
                    nc.gpsimd.tensor_mul(m1[:], zc[:], zc[:])
                    z2c = acts.tile([S, CH], BF16, tag="z2c")
                    nc.gpsimd.tensor_scalar(z2c[:], m1[:], -1.0, 1.0,
                                            mybir.AluOpType.mult, mybir.AluOpType.add)
                    m3 = acts.tile([S, CH], BF16, tag="m3")
                    nc.gpsimd.tensor_mul(m3[:], z2c[:], z2c[:])
                    cur = m3
                hf = outp.tile([S, DCH], BF16, tag="h")
                nc.vector.memset(hf, 0.0)
                nc.vector.tensor_copy(hf[:, 0:CH], cur[:])
                nc.sync.dma_start(out=outT[:, 0:DCH], in_=hf[:])

            if VARIANT == "dvechain":
                cur = wp.tile([S, CH], BF16, tag="xs")
                nc.vector.memset(cur, 0.25)
                for it in range(NCH * repeat):
                    for j in range(9):
                        tt = acts.tile([S, CH], BF16, tag=f"e{j}")
                        nc.vector.tensor_mul(tt[:], cur[:], cur[:])
                        cur = tt
                hf = outp.tile([S, DCH], BF16, tag="h")
                nc.vector.memset(hf, 0.0)
                nc.vector.tensor_copy(hf[:, 0:CH], cur[:])
                nc.sync.dma_start(out=outT[:, 0:DCH], in_=hf[:])

            if VARIANT in ("act2bank", "act1bank"):
                cur = wp.tile([S, 2 * CH], BF16, tag="xs")
                nc.vector.memset(cur, 0.25)
                for it in range(NCH * repeat):
                    ps_zr = psp.tile([S, 2 * CH], F32, tag="ps_zr")
                    nc.tensor.matmul(ps_zr[:, :CH], w_x1[0][:], cur[:, :CH], start=True, stop=False)
                    nc.tensor.matmul(ps_zr[:, :CH], w_h[0][:], cur[:, CH:], start=False, stop=True)
                    nc.tensor.matmul(ps_zr[:, CH:], w_x1[1][:], cur[:, :CH], start=True, stop=False)
                    nc.tensor.matmul(ps_zr[:, CH:], w_h[1][:], cur[:, CH:], start=False, stop=True)
                    zr = acts.tile([S, 2 * CH], BF16, tag="zr")
                    if VARIANT == "act2bank":
                        nc.scalar.activation(zr[:], ps_zr[:], AF.Sigmoid)
                    else:
                        nc.scalar.activation(zr[:, :CH], ps_zr[:, :CH], AF.Sigmoid)
                        nc.scalar.activation(zr[:, CH:], ps_zr[:, CH:], AF.Sigmoid)
                    cur = zr
                hf = outp.tile([S, DCH], BF16, tag="h")
                nc.vector.memset(hf, 0.0)
                nc.vector.tensor_copy(hf[:, 0:CH], cur[:, :CH])
                nc.sync.dma_start(out=outT[:, 0:DCH], in_=hf[:])

            if VARIANT == "gps4":
                xs = wp.tile([S, CH], BF16, tag="xs")
                nc.vector.memset(xs, 0.25)
                for it in range(NCH * repeat):
                    zc = acts.tile([S, CH], BF16, tag="zc")
                    nc.gpsimd.tensor_scalar(zc[:], xs[:], -1.0, 1.0,
                                            mybir.AluOpType.mult, mybir.AluOpType.add)
                    m1 = acts.tile([S, CH], BF16, tag="m1")
                    nc.gpsimd.tensor_mul(m1[:], xs[:], xs[:])
                    z2c = acts.tile([S, CH], BF16, tag="z2c")
                    nc.gpsimd.tensor_scalar(z2c[:], xs[:], -1.0, 1.0,
                                            mybir.AluOpType.mult, mybir.AluOpType.add)
                    m3 = acts.tile([S, CH], BF16, tag="m3")
                    nc.gpsimd.tensor_mul(m3[:], xs[:], xs[:])
                hf = outp.tile([S, DCH], BF16, tag="h")
                nc.vector.memset(hf, 0.0)
                nc.sync.dma_start(out=outT[:, 0:DCH], in_=hf[:])

            if VARIANT == "dve9":
                xs = wp.tile([S, CH], BF16, tag="xs")
                nc.vector.memset(xs, 0.25)
                for it in range(NCH * repeat):
                    for j in range(9):
                        tt = acts.tile([S, CH], BF16, tag=f"e{j}")
                        nc.vector.tensor_mul(tt[:], xs[:], xs[:])
                hf = outp.tile([S, DCH], BF16, tag="h")
                nc.vector.memset(hf, 0.0)
                nc.sync.dma_start(out=outT[:, 0:DCH], in_=hf[:])

            if VARIANT == "mm":
                xs = wp.tile([S, CH], BF16, tag="xs")
                nc.vector.memset(xs, 0.25)
                ohs = wp.tile([S, CH], BF16, tag="ohs")
                nc.vector.memset(ohs, 0.25)
                for it in range(NCH * repeat):
                    ps_zr = psp.tile([S, 2 * CH], F32, tag="ps_zr")
                    nc.tensor.matmul(ps_zr[:, :CH], w_x1[0][:], xs[:], start=True, stop=False)
                    nc.tensor.matmul(ps_zr[:, :CH], w_h[0][:], ohs[:], start=False, stop=True)
                    nc.tensor.matmul(ps_zr[:, CH:], w_x1[1][:], xs[:], start=True, stop=False)
                    nc.tensor.matmul(ps_zr[:, CH:], w_h[1][:], ohs[:], start=False, stop=True)
                    ps_ht = psp2.tile([S, CH], F32, tag="ps_ht")
                    nc.tensor.matmul(ps_ht[:], w_x1[2][:], xs[:], start=True, stop=False)
                    nc.tensor.matmul(ps_ht[:], w_h[2][:], ohs[:], start=False, stop=True)
                    ps_mid = psp.tile([S, CH], F32, tag="ps_mid")
                    nc.tensor.matmul(ps_mid[:], w_mid[:], ohs[:], start=True, stop=True)
                    ps_zr2 = psp.tile([S, 2 * CH], F32, tag="ps_zr2")
                    nc.tensor.matmul(ps_zr2[:, :CH], w_x2[0][:], xs[:], start=True, stop=False)
                    nc.tensor.matmul(ps_zr2[:, :CH], w_h[0][:], ohs[:], start=False, stop=True)
                    nc.tensor.matmul(ps_zr2[:, CH:], w_x2[1][:], xs[:], start=True, stop=False)
                    nc.tensor.matmul(ps_zr2[:, CH:], w_h[4][:], ohs[:], start=False, stop=True)
                    ps_ht2 = psp.tile([S, CH], F32, tag="ps_ht2")
                    nc.tensor.matmul(ps_ht2[:], w_x2[2][:], xs[:], start=True, stop=False)
                    nc.tensor.matmul(ps_ht2[:], w_h[5][:], ohs[:], start=False, stop=True)
                hf = outp.tile([S, DCH], BF16, tag="h")
                nc.vector.memset(hf, 0.0)
                nc.sync.dma_start(out=outT[:, 0:DCH], in_=hf[:])

            if VARIANT == "mmact":
                xs = wp.tile([S, CH], BF16, tag="xs")
                nc.vector.memset(xs, 0.25)
                ohs = wp.tile([S, CH], BF16, tag="ohs")
                nc.vector.memset(ohs, 0.25)
                for it in range(NCH * repeat):
                    ps_zr = psp.tile([S, 2 * CH], F32, tag="ps_zr")
                    nc.tensor.matmul(ps_zr[:, :CH], w_x1[0][:], xs[:], start=True, stop=False)
                    nc.tensor.matmul(ps_zr[:, :CH], w_h[0][:], ohs[:], start=False, stop=True)
                    nc.tensor.matmul(ps_zr[:, CH:], w_x1[1][:], xs[:], start=True, stop=False)
                    nc.tensor.matmul(ps_zr[:, CH:], w_h[1][:], ohs[:], start=False, stop=True)
                    zr = acts.tile([S, 2 * CH], BF16, tag="zr")
                    nc.scalar.activation(zr[:], ps_zr[:], AF.Sigmoid)
                    ps_ht = psp2.tile([S, CH], F32, tag="ps_ht")
                    nc.tensor.matmul(ps_ht[:], w_x1[2][:], xs[:], start=True, stop=False)
                    nc.tensor.matmul(ps_ht[:], w_h[2][:], ohs[:], start=False, stop=True)
                    ht = acts.tile([S, CH], BF16, tag="ht")
                    nc.scalar.activation(ht[:], ps_ht[:], AF.Tanh)
                    ps_mid = psp.tile([S, CH], F32, tag="ps_mid")
                    nc.tensor.matmul(ps_mid[:], w_mid[:], ohs[:], start=True, stop=True)
                    mx = acts.tile([S, CH], BF16, tag="mid_x")
                    nc.vector.tensor_scalar_max(mx[:], ps_mid[:], 0.0)
                    ps_zr2 = psp.tile([S, 2 * CH], F32, tag="ps_zr2")
                    nc.tensor.matmul(ps_zr2[:, :CH], w_x2[0][:], xs[:], start=True, stop=False)
                    nc.tensor.matmul(ps_zr2[:, :CH], w_h[0][:], ohs[:], start=False, stop=True)
                    nc.tensor.matmul(ps_zr2[:, CH:], w_x2[1][:], xs[:], start=True, stop=False)
                    nc.tensor.matmul(ps_zr2[:, CH:], w_h[4][:], ohs[:], start=False, stop=True)
                    zr2 = acts.tile([S, 2 * CH], BF16, tag="zr2")
                    nc.scalar.activation(zr2[:], ps_zr2[:], AF.Sigmoid)
                    ps_ht2 = psp.tile([S, CH], F32, tag="ps_ht2")
                    nc.tensor.matmul(ps_ht2[:], w_x2[2][:], xs[:], start=True, stop=False)
                    nc.tensor.matmul(ps_ht2[:], w_h[5][:], ohs[:], start=False, stop=True)
                    ht2 = acts.tile([S, CH], BF16, tag="ht2")
                    nc.scalar.activation(ht2[:], ps_ht2[:], AF.Tanh)
                hf = outp.tile([S, DCH], BF16, tag="h")
                nc.vector.memset(hf, 0.0)
                nc.sync.dma_start(out=outT[:, 0:DCH], in_=hf[:])

            if VARIANT == "full":
                # group-granular 2-stage pipeline over groups of 4 chunks;
                # ACT calls pair-width, GRU combines quad-width. Per-pair
                # phase order is chosen so each in-order queue never blocks:
                # DVE sees relu (old deps) before rh/r2h (fresh deps), and
                # every ACT tanh has >=1.15us of independent sigmoid work
                # interposed after the sigmoid its input chains from.
                PAIR = 2 * CH
                QUAD = 4 * CH
                NG = NCH // GRP
                st1 = {}
                st2 = {}
                dmas = {}

                def dma_group(g):
                    gd = g % NG
                    gs = bass.ts(gd, DCH)
                    x4 = inp.tile([S, DCH], BF16, tag="x")
                    nc.sync.dma_start(out=x4, in_=xT[:, gs])
                    oh4 = inp.tile([S, DCH], BF16, tag="oh")
                    nc.sync.dma_start(out=oh4, in_=ohT[:, gs])
                    dmas[g] = (gs, x4, oh4)

                def prs(q):
                    u = 2 * q
                    pr = slice(u * CH, (u + 2) * CH)
                    cs = [slice((u + k) * CH, (u + k + 1) * CH) for k in (0, 1)]
                    return pr, cs

                def s1_sig(g, q):
                    if q == 0:
                        gs, x4, oh4 = dmas.pop(g)
                        zq = acts.tile([S, QUAD], BF16, tag="zq")
                        rq = acts.tile([S, QUAD], BF16, tag="rq")
                        htq = acts.tile([S, QUAD], BF16, tag="htq")
                        st1[g] = dict(gs=gs, x4=x4, oh4=oh4, zq=zq, rq=rq,
                                      htq=htq, rh=[None, None])
                    s = st1[g]
                    x4, oh4 = s["x4"], s["oh4"]
                    pr, cs = prs(q)

                    pza = psp.tile([S, PAIR], F32, tag="pza")
                    for k in (0, 1):
                        o = slice(k * CH, (k + 1) * CH)
                        nc.tensor.matmul(pza[:, o], w_x1[0][:], x4[:, cs[k]], start=True, stop=False)
                        nc.tensor.matmul(pza[:, o], w_h[0][:], oh4[:, cs[k]], start=False, stop=True)
                    act(s["zq"][:, pr], pza[:], AF.Sigmoid, 0)

                    pzb = psp.tile([S, PAIR], F32, tag="pzb")
                    for k in (0, 1):
                        o = slice(k * CH, (k + 1) * CH)
                        nc.tensor.matmul(pzb[:, o], w_x1[1][:], x4[:, cs[k]], start=True, stop=False)
                        nc.tensor.matmul(pzb[:, o], w_h[1][:], oh4[:, cs[k]], start=False, stop=True)
                    act(s["rq"][:, pr], pzb[:], AF.Sigmoid, 1)

                    rh = scr.tile([S, PAIR], BF16, tag="rh")
                    nc.vector.tensor_mul(rh[:], s["rq"][:, pr], oh4[:, pr])
                    s["rh"][q] = rh

                def s1_tanh(g, q):
                    s = st1[g]
                    x4, rh = s["x4"], s["rh"][q]
                    pr, cs = prs(q)
                    pt = psp.tile([S, PAIR], F32, tag="pt")
                    for k in (0, 1):
                        o = slice(k * CH, (k + 1) * CH)
                        nc.tensor.matmul(pt[:, o], w_x1[2][:], x4[:, cs[k]], start=True, stop=False)
                        nc.tensor.matmul(pt[:, o], w_h[2][:], rh[:, o], start=False, stop=True)
                    act(s["htq"][:, pr], pt[:], AF.Tanh, 2)

                def s1_quad(g):
                    s = st1.pop(g)
                    oh4, zq, htq = s["oh4"], s["zq"], s["htq"]
                    dq = scr.tile([S, QUAD], BF16, tag="dq")
                    nc.vector.tensor_sub(dq[:], oh4[:], htq[:])
                    pq = scr.tile([S, QUAD], BF16, tag="pq")
                    nc.vector.tensor_mul(pq[:], zq[:], dq[:])
                    mid_hq = acts.tile([S, QUAD], BF16, tag="mid_hq")
                    nc.vector.tensor_add(mid_hq[:], htq[:], pq[:])
                    st2[g] = dict(gs=s["gs"], mid_hq=mid_hq, r2h=[None, None])

                def s2_mid_sig(g, q):
                    s = st2[g]
                    if q == 0:
                        mxq = acts.tile([S, QUAD], BF16, tag="mxq")
                        z2q = acts.tile([S, QUAD], BF16, tag="z2q")
                        r2q = acts.tile([S, QUAD], BF16, tag="r2q")
                        ht2q = acts.tile([S, QUAD], BF16, tag="ht2q")
                        s.update(mxq=mxq, z2q=z2q, r2q=r2q, ht2q=ht2q)
                    mid_hq, mxq = s["mid_hq"], s["mxq"]
                    pr, cs = prs(q)

                    pm = psp.tile([S, PAIR], F32, tag="pm")
                    for k in (0, 1):
                        o = slice(k * CH, (k + 1) * CH)
                        nc.tensor.matmul(pm[:, o], w_mid[:], mid_hq[:, cs[k]], start=True, stop=True)
                    nc.vector.tensor_scalar_max(mxq[:, pr], pm[:], 0.0)

                    pza = psp.tile([S, PAIR], F32, tag="pza")
                    for k in (0, 1):
                        o = slice(k * CH, (k + 1) * CH)
                        nc.tensor.matmul(pza[:, o], w_x2[0][:], mxq[:, cs[k]], start=True, stop=False)
                        nc.tensor.matmul(pza[:, o], w_h[0][:], mid_hq[:, cs[k]], start=False, stop=True)
                    act(s["z2q"][:, pr], pza[:], AF.Sigmoid, 0)

                    pzb = psp.tile([S, PAIR], F32, tag="pzb")
                    for k in (0, 1):
                        o = slice(k * CH, (k + 1) * CH)
                        nc.tensor.matmul(pzb[:, o], w_x2[1][:], mxq[:, cs[k]], start=True, stop=False)
                        nc.tensor.matmul(pzb[:, o], w_h[4][:], mid_hq[:, cs[k]], start=False, stop=True)
                    act(s["r2q"][:, pr], pzb[:], AF.Sigmoid, 4)

                    r2h = scr.tile([S, PAIR], BF16, tag="r2h")
                    nc.vector.tensor_mul(r2h[:], s["r2q"][:, pr], mid_hq[:, pr])
                    s["r2h"][q] = r2h

                def s2_tanh(g, q):
                    s = st2[g]
                    mxq, r2h = s["mxq"], s["r2h"][q]
                    pr, cs = prs(q)
                    pt = psp.tile([S, PAIR], F32, tag="pt")
                    for k in (0, 1):
                        o = slice(k * CH, (k + 1) * CH)
                        nc.tensor.matmul(pt[:, o], w_x2[2][:], mxq[:, cs[k]], start=True, stop=False)
                        nc.tensor.matmul(pt[:, o], w_h[5][:], r2h[:, o], start=False, stop=True)
                    act(s["ht2q"][:, pr], pt[:], AF.Tanh, 5)

                def s2_quad(g):
                    s = st2.pop(g)
                    mid_hq, z2q, ht2q = s["mid_hq"], s["z2q"], s["ht2q"]
                    d2q = scr.tile([S, QUAD], BF16, tag="d2q")
                    nc.vector.tensor_sub(d2q[:], mid_hq[:], ht2q[:])
                    p2q = scr.tile([S, QUAD], BF16, tag="p2q")
                    nc.vector.tensor_mul(p2q[:], z2q[:], d2q[:])
                    h4 = outp.tile([S, QUAD], BF16, tag="h")
                    nc.vector.tensor_add(h4[:], ht2q[:], p2q[:])
                    nc.sync.dma_start(out=outT[:, s["gs"]], in_=h4[:])

                NGT = NG * repeat
                dma_group(0)
                if NGT > 1:
                    dma_group(1)
                for q in (0, 1):
                    s1_sig(0, q)
                    s1_tanh(0, q)
                s1_quad(0)
                for g in range(NGT):
                    nxt = g + 1 < NGT
                    for q in (0, 1):
                        s2_mid_sig(g, q)
                        if nxt:
                            s1_sig(g + 1, q)
                        s2_tanh(g, q)
                        if nxt:
                            s1_tanh(g + 1, q)
                    if nxt:
                        if g + 2 < NGT:
                            dma_group(g + 2)
                        s1_quad(g + 1)
                    s2_quad(g)

    if compile:
        nc.compile()
    return nc


def _prep_in_maps(x, old_h, W_x1, W_x2, W_h, b, mid):
    """Host-side sharding + layout + dtype prep. Returns per-core input maps."""
    x = np.asarray(x, dtype=np.float32)
    old_h = np.asarray(old_h, dtype=np.float32)
    W_x1 = np.ascontiguousarray(W_x1, dtype=np.float32).astype(NPBF16)
    W_x2 = np.ascontiguousarray(W_x2, dtype=np.float32).astype(NPBF16)
    W_h = np.ascontiguousarray(W_h, dtype=np.float32).astype(NPBF16)
    b = np.asarray(b, dtype=np.float32)
    mid = np.ascontiguousarray(mid, dtype=np.float32).astype(NPBF16)

    xT = np.ascontiguousarray(x.T).astype(NPBF16)      # [S, B]
    ohT = np.ascontiguousarray(old_h.T).astype(NPBF16)
    bTh = np.ascontiguousarray(b.reshape(6, 1, S).transpose(0, 2, 1))  # [6,S,1]

    in_maps = []
    for c in range(NCORES):
        sl = slice(c * BC, (c + 1) * BC)
        in_maps.append({
            "xT": np.ascontiguousarray(xT[:, sl]),
            "ohT": np.ascontiguousarray(ohT[:, sl]),
            "wx1": W_x1,
            "wx2": W_x2,
            "wh": W_h,
            "bT": bTh,
            "midw": mid,
        })
    return in_maps


def kernel(x, old_h, W_x1, W_x2, W_h, b, mid, trace=False):
    assert np.asarray(x).shape == (B, S) and np.asarray(old_h).shape == (B, S)
    b = np.asarray(b, dtype=np.float32)
    use_bias = bool(np.any(b != 0.0))
    key = use_bias
    if key not in _NC_CACHE:
        _NC_CACHE[key] = _build(use_bias)
    nc = _NC_CACHE[key]

    in_maps = _prep_in_maps(x, old_h, W_x1, W_x2, W_h, b, mid)
    res = bass_utils.run_bass_kernel_spmd(
        nc, in_maps, core_ids=list(range(NCORES)), trace=trace
    )
    outT = np.concatenate(
        [res.results[c]["outT"].astype(np.float32) for c in range(NCORES)], axis=1
    )
    h = np.ascontiguousarray(outT.T)
    if trace:
        return (h,), res
    return (h,)
